# revision 1
# baseline (speedup 1.0000x reference)
"""GIN (3-layer) Trainium2 Bass kernel, 8-core SPMD.  v4

Sharding: nodes (and incident edges, by dst) partitioned across 8 cores;
segment_sum local per dst shard; features exchanged between layers with a
row-split AllGather (two waves) overlapped with the MLP; MLP weights
replicated.

Key mechanics:
  - per-core dedup of gather rows at 256-dst granularity (pair-block groups);
    indirect dma_gather pulls fp8 rows (values/16) from HBM, split into an
    A-wave (rows 0-511 of every shard) and a B-wave (rows 512-1279) so the
    next layer's gather can start when the first AllGather wave lands.
  - segment-sum on the PE in fp8 DoubleRow mode: chunk-pair matmuls with a
    host-built one-hot selector S (entries 16*mult, resident in SBUF).
  - agg evacuated to bf16 (DVE), transposed on the PE, added to resident bf16
    h^T; 2-layer MLP feature-major bf16; bias+ReLU fused on scalar.
  - h^T transposed back per block, scaled 1/16 to fp8: blocks 0-3 ship and
    AllGather (wave A) while the MLP finishes, then blocks 4-9 (wave B).
"""

import os
import sys
from contextlib import ExitStack

import numpy as np

for _p in ("/opt/trn_rl_repo", "/root/.axon_site/_ro/trn_rl_repo"):
    if os.path.isdir(_p) and _p not in sys.path:
        sys.path.append(_p)

import ml_dtypes

N_NODES = 10000
N_EDGES = 160000
D = 512
N_LAYERS = 3
CORES = 8
SHARD = N_NODES // CORES          # 1250 nodes per core
PADS = 1280                       # padded shard (multiple of 128)
PADN = CORES * PADS               # padded full node count (10240)
NB = PADS // 128                  # dst blocks per core (10)
NG = NB // 2                      # dedup groups (256 dsts each)
ASPL = 512                        # A-wave rows per shard (blocks 0-3)
BSPL = PADS - ASPL                # B-wave rows per shard (768, blocks 4-9)
FSCALE = 16.0                     # fp8 feature scale (folded into S)

BF16 = ml_dtypes.bfloat16
F8 = ml_dtypes.float8_e4m3fn

LAST_RESULTS = None


def _even_ceil(n):
    return max(2, -(-n // 128) + (-(-n // 128) & 1))


def _prep_host(x, edge_index, Ws, bs):
    x = np.asarray(x, np.float32)
    src = np.asarray(edge_index[0], np.int64)
    dst = np.asarray(edge_index[1], np.int64)
    Ws = np.asarray(Ws, np.float32)
    bs = np.asarray(bs, np.float32)

    g = (src // SHARD) * PADS + (src % SHARD)   # padded global src row
    sh = g // PADS
    r = g - sh * PADS
    is_b = r >= ASPL
    row_ab = np.where(is_b, sh * BSPL + (r - ASPL), sh * ASPL + r)  # idx in wave buf

    owner = dst // SHARD
    dloc = dst - owner * SHARD
    grp = dloc // 256
    blk_in = (dloc % 256) // 128
    j = dloc % 128

    # Uniform per-group chunk counts (max over cores), even for DR pairing.
    CA = np.zeros(NG, np.int64)
    CB = np.zeros(NG, np.int64)
    for c in range(CORES):
        for q in range(NG):
            m = (owner == c) & (grp == q)
            u = np.unique(g[m])
            nA = int((u % PADS < ASPL).sum())
            nB = len(u) - nA
            CA[q] = max(CA[q], _even_ceil(nA))
            CB[q] = max(CB[q], _even_ceil(nB))
    C_A = [int(v) for v in CA]
    C_B = [int(v) for v in CB]
    C_tot = [a + b for a, b in zip(C_A, C_B)]
    CMAX = max(C_tot)
    PB = CMAX // 2

    # fp8 wave gather sources for layer 0 (x/16).
    xa = np.zeros((CORES * ASPL, D), F8)
    xb = np.zeros((CORES * BSPL, D), F8)
    for o in range(CORES):
        xs = (x[o * SHARD:(o + 1) * SHARD] / FSCALE).astype(F8)  # [1250, D]
        xa[o * ASPL:o * ASPL + ASPL] = xs[:ASPL]
        xb[o * BSPL:o * BSPL + SHARD - ASPL] = xs[ASPL:]

    Wd = np.ascontiguousarray(Ws.reshape(2 * N_LAYERS, D, D).astype(BF16))
    bT = np.ascontiguousarray(
        bs.reshape(2 * N_LAYERS, 4, 128).transpose(2, 0, 1).reshape(128, 8 * N_LAYERS))
    identb = np.eye(128, dtype=BF16)

    in_maps = []
    for c in range(CORES):
        Scnt = np.zeros((128, NG * 2 * PB * 2, 128), np.int16)
        idxd = np.zeros((128, NG * CMAX * 8), np.int16)
        for q in range(NG):
            m = (owner == c) & (grp == q)
            eg = g[m]
            uniq, inv = np.unique(eg, return_inverse=True)
            ub = uniq % PADS >= ASPL
            nA = int((~ub).sum())
            nB = len(uniq) - nA
            cA, cB = C_A[q], C_B[q]
            posmap = np.empty(len(uniq), np.int64)
            posmap[~ub] = np.arange(nA)
            posmap[ub] = cA * 128 + np.arange(nB)
            pos = posmap[inv]
            cc = pos // 128
            blk = blk_in[m]
            np.add.at(Scnt,
                      (pos % 128,
                       ((q * 2 + blk) * PB + cc // 2) * 2 + cc % 2,
                       j[m]), 1)
            glist = np.zeros((cA + cB) * 128, np.int16)
            u_rows = np.where(ub, (uniq // PADS) * BSPL + (uniq % PADS - ASPL),
                              (uniq // PADS) * ASPL + uniq % PADS).astype(np.int16)
            glist[:nA] = u_rows[~ub]
            glist[cA * 128:cA * 128 + nB] = u_rows[ub]
            C = cA + cB
            w = glist.reshape(C * 8, 16).T
            idxd[:, q * CMAX * 8:q * CMAX * 8 + C * 8] = np.tile(w, (8, 1))
        Sd = (Scnt.astype(np.float32) * FSCALE).astype(F8)
        # Pre-gathered layer-0 chunks (host-side gather; no desc-gen on device).
        segs = []
        for q in range(NG):
            cA, cB = C_A[q], C_B[q]
            C = cA + cB
            w = idxd[:16, q * CMAX * 8:q * CMAX * 8 + C * 8]
            gl = w.T.reshape(C * 128).astype(np.int64)
            rows = np.empty((C * 128, D), F8)
            rows[:cA * 128] = xa[gl[:cA * 128]]
            rows[cA * 128:] = xb[gl[cA * 128:]]
            segs.append(rows.reshape(C, 128, D).transpose(1, 0, 2).reshape(128, C * D))
        xgc = np.ascontiguousarray(np.concatenate(segs, axis=1))
        xT_own = np.zeros((D, PADS), np.float32)
        xT_own[:, :SHARD] = x[c * SHARD:(c + 1) * SHARD].T
        in_maps.append({
            "xgc": xgc,
            "xa": xa,
            "xb": xb,
            "xT": xT_own.astype(BF16),
            "Wd": Wd,
            "bT": bT,
            "identb": identb,
            "Sd": Sd,
            "idxd": idxd,
        })
    return in_maps, C_A, C_B, CMAX


def build_program(C_A, C_B, CMAX):
    import concourse.bacc as bacc
    import concourse.bass as bass
    import concourse.mybir as mybir
    import concourse.tile as tile

    dt = mybir.dt
    f32, bf16, f8, i16 = dt.float32, dt.bfloat16, dt.float8e4, dt.int16
    AF = mybir.ActivationFunctionType
    DR = mybir.MatmulPerfMode.DoubleRow
    PB = CMAX // 2
    CHMAX = max(C_A + C_B)

    nc = bacc.Bacc("TRN2", target_bir_lowering=False, debug=False,
                   enable_asserts=False, num_devices=CORES, num_swdge_queues=4)

    TOTC = sum(C_A) + sum(C_B)
    xgc = nc.dram_tensor("xgc", [128, TOTC * D], f8, kind="ExternalInput")
    xa = nc.dram_tensor("xa", [CORES * ASPL, D], f8, kind="ExternalInput")
    xb = nc.dram_tensor("xb", [CORES * BSPL, D], f8, kind="ExternalInput")
    xT = nc.dram_tensor("xT", [D, PADS], bf16, kind="ExternalInput")
    Wd = nc.dram_tensor("Wd", [2 * N_LAYERS, D, D], bf16, kind="ExternalInput")
    bTd = nc.dram_tensor("bT", [128, 8 * N_LAYERS], f32, kind="ExternalInput")
    identbd = nc.dram_tensor("identb", [128, 128], bf16, kind="ExternalInput")
    Sdr = nc.dram_tensor("Sd", [128, NG * 2 * PB * 2, 128], f8, kind="ExternalInput")
    idxd = nc.dram_tensor("idxd", [128, NG * CMAX * 8], i16, kind="ExternalInput")
    outTd = nc.dram_tensor("outT", [D, PADS], f32, kind="ExternalOutput")

    NCHUNK = [(0, 512), (512, 512), (1024, PADS - 1024)]

    with tile.TileContext(nc) as tc, ExitStack() as ctx:
        p_const = ctx.enter_context(tc.tile_pool(name="const", bufs=1))
        p_big = ctx.enter_context(tc.tile_pool(name="big", bufs=1))
        p_g = ctx.enter_context(tc.tile_pool(name="gth", bufs=12))
        p_w = ctx.enter_context(tc.tile_pool(name="wts", bufs=2))
        p_aggn = ctx.enter_context(tc.tile_pool(name="aggn", bufs=4))
        p_hbf = ctx.enter_context(tc.tile_pool(name="hbf", bufs=2))
        p_ot = ctx.enter_context(tc.tile_pool(name="ot", bufs=2))
        p_aggps = ctx.enter_context(tc.tile_pool(name="aggps", bufs=4, space="PSUM"))
        p_tps = ctx.enter_context(tc.tile_pool(name="tps", bufs=2, space="PSUM"))
        p_mlpps = ctx.enter_context(tc.tile_pool(name="mlpps", bufs=2, space="PSUM"))
        p_dram = ctx.enter_context(tc.tile_pool(name="dram", bufs=1, space="DRAM"))

        idxs = p_const.tile([128, NG * CMAX * 8], i16)
        nc.sync.dma_start(idxs[:], idxd.ap())
        identb = p_const.tile([128, 128], bf16)
        nc.sync.dma_start(identb[:], identbd.ap())
        bt = p_const.tile([128, 8 * N_LAYERS], f32)
        nc.sync.dma_start(bt[:], bTd.ap())

        S = p_big.tile([128, NG * 2 * PB * 2, 128], f8)
        NSL = NG * 2
        SW = PB * 2
        for s in range(NSL):
            nc.scalar.dma_start(S[:, s * SW:(s + 1) * SW, :],
                                Sdr.ap()[:, s * SW:(s + 1) * SW, :])

        hT = p_big.tile([128, 4, PADS], bf16)
        ZT = p_big.tile([128, 4, PADS], bf16)
        Y1T = p_big.tile([128, 4, PADS], bf16)
        for kc in range(4):
            nc.sync.dma_start(hT[:, kc, :], xT.ap()[kc * 128:(kc + 1) * 128, :])

        wa_in = p_dram.tile([128, D], bf16, name="wa_in")
        wa_out = p_dram.tile([128 * CORES, D], bf16, addr_space="Shared", name="wa_out")
        nc.sync.dma_start(wa_in[:, :], xT.ap()[0:128, 0:D])

        hsh_a = [p_dram.tile([ASPL, D], f8, name=f"hsa{l}") for l in range(2)]
        hsh_b = [p_dram.tile([BSPL, D], f8, name=f"hsb{l}") for l in range(2)]
        ag_a = [p_dram.tile([CORES * ASPL, D], f8, addr_space="Shared",
                            name=f"aga{l}") for l in range(2)]
        ag_b = [p_dram.tile([CORES * BSPL, D], f8, addr_space="Shared",
                            name=f"agb{l}") for l in range(2)]

        qctr = [0]
        GOFS = []  # xgc column offset of each (group, half)
        _o = 0
        for _q in range(NG):
            GOFS.append((_o, _o + C_A[_q] * D))
            _o += (C_A[_q] + C_B[_q]) * D

        def emit_gather(q, half, gsrc):
            C = C_A[q] if half == 0 else C_B[q]
            g = p_g.tile([128, CHMAX, D], f8, tag="g", name="g")
            if gsrc is None:  # layer 0: host pre-gathered, contiguous load
                o = GOFS[q][half]
                nc.sync.dma_start(g[:, :C, :], xgc.ap()[:, o:o + C * D])
                return g
            qn = qctr[0] % 4
            qctr[0] += 1
            base = q * CMAX * 8 + (0 if half == 0 else C_A[q] * 8)
            nc.gpsimd.dma_gather(
                out_ap=g[:, :C, :],
                in_ap=gsrc,
                idxs_ap=idxs[:, base:base + C * 8],
                num_idxs=C * 128,
                num_idxs_reg=C * 128,
                elem_size=D,
                single_packet=False,
                queue_num=qn,
            )
            return g

        def emit_epi(l, b):
            hb = p_hbf.tile([128, D], f8, tag="hbf", name="hb")
            for fc in range(4):
                pt2 = p_tps.tile([128, 128], bf16, tag="t", name="pt2")
                nc.tensor.transpose(pt2[:], hT[:, fc, b * 128:(b + 1) * 128],
                                    identb[:])
                nc.scalar.activation(hb[:, fc * 128:(fc + 1) * 128], pt2[:],
                                     AF.Identity, scale=1.0 / FSCALE)
            if b < 4:
                nc.scalar.dma_start(hsh_a[l][b * 128:(b + 1) * 128, :], hb[:])
            else:
                nc.scalar.dma_start(hsh_b[l][(b - 4) * 128:(b - 3) * 128, :], hb[:])

        def emit_mlp_chunk(l, j, nofs, nw):
            rhs_big = ZT if j == 0 else Y1T
            Wt = Wts[l][j]
            for mc in range(4):
                ps2 = p_mlpps.tile([128, D], f32, tag="mlp", name="ps2")
                for kc in range(4):
                    nc.tensor.matmul(
                        ps2[:, :nw],
                        lhsT=Wt[:, kc, mc * 128:(mc + 1) * 128],
                        rhs=rhs_big[:, kc, nofs:nofs + nw],
                        start=(kc == 0), stop=(kc == 3))
                col = (2 * l + j) * 4 + mc
                bias = bt[:, col:col + 1]
                if j == 0:
                    nc.scalar.activation(Y1T[:, mc, nofs:nofs + nw],
                                         ps2[:, :nw], AF.Relu, bias=bias)
                elif l < N_LAYERS - 1:
                    nc.scalar.activation(hT[:, mc, nofs:nofs + nw],
                                         ps2[:, :nw], AF.Relu, bias=bias)
                else:
                    ot = p_ot.tile([128, 512], f32, tag="ot", name="ot")
                    nc.scalar.activation(ot[:, :nw], ps2[:, :nw],
                                         AF.Identity, bias=bias)
                    nc.sync.dma_start(
                        outTd.ap()[mc * 128:(mc + 1) * 128, nofs:nofs + nw],
                        ot[:, :nw])

        Wts = {}
        for l in range(N_LAYERS):
            srcA = None if l == 0 else ag_a[l - 1][:, :]
            srcB = None if l == 0 else ag_b[l - 1][:, :]

            W0t = p_w.tile([128, 4, D], bf16, tag="w", name="W0t")
            W1t = p_w.tile([128, 4, D], bf16, tag="w", name="W1t")
            Wts[l] = (W0t, W1t)
            for kc in range(4):
                nc.sync.dma_start(W0t[:, kc, :], Wd.ap()[2 * l, kc * 128:(kc + 1) * 128, :])
                nc.sync.dma_start(W1t[:, kc, :], Wd.ap()[2 * l + 1, kc * 128:(kc + 1) * 128, :])

            # All gathers of the layer up-front: A-wave first, then B-wave.
            gA = [emit_gather(q, 0, srcA) for q in range(NG)]
            gB = [emit_gather(q, 1, srcB) for q in range(NG)]
            if l == 0:
                nc.gpsimd.collective_compute(
                    "AllGather", mybir.AluOpType.bypass,
                    replica_groups=[list(range(CORES))],
                    ins=[wa_in.opt()], outs=[wa_out.opt()])

            def emit_agg(q):
                cA, cB = C_A[q], C_B[q]
                NP = (cA + cB) // 2
                ps = [p_aggps.tile([128, D], f32, tag="agg", name="ps")
                      for _ in range(2)]
                for p in range(NP):
                    if 2 * p < cA:
                        rhs = gA[q][:, 2 * p:2 * p + 2, :]
                    else:
                        o = 2 * p - cA
                        rhs = gB[q][:, o:o + 2, :]
                    for blk in range(2):
                        base = ((q * 2 + blk) * PB + p) * 2
                        nc.tensor.matmul(ps[blk][:], lhsT=S[:, base:base + 2, :],
                                         rhs=rhs,
                                         start=(p == 0), stop=(p == NP - 1),
                                         perf_mode=DR)
                for blk in range(2):
                    b = q * 2 + blk
                    aggN = p_aggn.tile([128, D], bf16, name="aggN")
                    nc.vector.tensor_copy(aggN[:], ps[blk][:])
                    for fc in range(4):
                        pt = p_tps.tile([128, 128], bf16, tag="t", name="pt")
                        nc.tensor.transpose(pt[:], aggN[:, fc * 128:(fc + 1) * 128],
                                            identb[:])
                        nc.vector.tensor_add(ZT[:, fc, b * 128:(b + 1) * 128], pt[:],
                                             hT[:, fc, b * 128:(b + 1) * 128])

            # ---- agg interleaved with MLP; split epilogue + split AllGather ----
            emit_agg(0)
            emit_agg(1)
            emit_mlp_chunk(l, 0, 0, 512)
            emit_agg(2)
            emit_agg(3)
            emit_mlp_chunk(l, 0, 512, 512)
            emit_agg(4)
            emit_mlp_chunk(l, 0, 1024, PADS - 1024)
            emit_mlp_chunk(l, 1, 0, 512)
            if l < N_LAYERS - 1:
                for b in range(4):
                    emit_epi(l, b)
                nc.gpsimd.collective_compute(
                    "AllGather", mybir.AluOpType.bypass,
                    replica_groups=[list(range(CORES))],
                    ins=[hsh_a[l].opt()], outs=[ag_a[l].opt()])
            emit_mlp_chunk(l, 1, 512, 512)
            emit_mlp_chunk(l, 1, 1024, PADS - 1024)
            if l < N_LAYERS - 1:
                for b in range(4, NB):
                    emit_epi(l, b)
                nc.gpsimd.collective_compute(
                    "AllGather", mybir.AluOpType.bypass,
                    replica_groups=[list(range(CORES))],
                    ins=[hsh_b[l].opt()], outs=[ag_b[l].opt()])

    nc.compile()
    return nc


def kernel(**inputs):
    global LAST_RESULTS
    from concourse import bass_utils

    in_maps, C_A, C_B, CMAX = _prep_host(
        inputs["x"], inputs["edge_index"], inputs["Ws"], inputs["bs"])
    nc = build_program(C_A, C_B, CMAX)
    res = bass_utils.run_bass_kernel_spmd(
        nc, in_maps, core_ids=list(range(CORES)),
        trace=bool(int(os.environ.get("GIN_TRACE", "0"))),
        tmpdir=os.environ.get("GIN_TMPDIR"),
    )
    LAST_RESULTS = res
    out = np.empty((N_NODES, D), np.float32)
    for c in range(CORES):
        out[c * SHARD:(c + 1) * SHARD] = res.results[c]["outT"][:, :SHARD].T
    return out



# revision 6
# speedup vs baseline: 1.0468x; 1.0468x over previous
"""GIN (3-layer) Trainium2 Bass kernel, 8-core SPMD.  v5

Sharding: nodes (and incident edges, by dst) partitioned across 8 cores;
segment_sum local per dst shard; features exchanged between layers with a
row-split AllGather (two waves) overlapped with the MLP; MLP weights
replicated.

v5 layout (vs v4):
  - dedup at 128-dst (block) granularity: ~158 fp8 chunks/layer/core, 79
    DoubleRow selector matmuls; chunk pairing crosses the A/B wave boundary
    (single even-ceil per block).
  - node-major resident h (bf16); Z = agg + h fused on DVE straight from
    PSUM; only Z is transposed (PE) for the feature-major MLP first GEMM.
  - second GEMM per node-block with Y1^T as the stationary operand ->
    node-major output directly (no epilogue transposes); bias via a K=1
    matmul row.
  - per-block indirect gathers (block-level agg dependency); collectives
    emitted before next layer's gathers on the gpsimd queue.
"""

import os
import sys
from contextlib import ExitStack

import numpy as np

for _p in ("/opt/trn_rl_repo", "/root/.axon_site/_ro/trn_rl_repo"):
    if os.path.isdir(_p) and _p not in sys.path:
        sys.path.append(_p)

import ml_dtypes

N_NODES = 10000
N_EDGES = 160000
D = 512
N_LAYERS = 3
CORES = 8
SHARD = N_NODES // CORES          # 1250 nodes per core
PADS = 1280                       # padded shard (multiple of 128)
NB = PADS // 128                  # dst blocks per core (10)
ASPL = 512                        # A-wave rows per shard (blocks 0-3)
BSPL = PADS - ASPL                # B-wave rows per shard (768, blocks 4-9)
FSCALE = 16.0                     # fp8 feature scale (folded into S)

BF16 = ml_dtypes.bfloat16
F8 = ml_dtypes.float8_e4m3fn

LAST_RESULTS = None


def _prep_host(x, edge_index, Ws, bs):
    x = np.asarray(x, np.float32)
    src = np.asarray(edge_index[0], np.int64)
    dst = np.asarray(edge_index[1], np.int64)
    Ws = np.asarray(Ws, np.float32)
    bs = np.asarray(bs, np.float32)

    g = (src // SHARD) * PADS + (src % SHARD)   # padded global src row
    owner = dst // SHARD
    dloc = dst - owner * SHARD
    blk = dloc // 128
    j = dloc % 128

    # Uniform per-block chunk counts (max over cores).
    CA = np.zeros(NB, np.int64)
    CB = np.zeros(NB, np.int64)
    for c in range(CORES):
        for b in range(NB):
            m = (owner == c) & (blk == b)
            u = np.unique(g[m])
            nA = int((u % PADS < ASPL).sum())
            nB = len(u) - nA
            CA[b] = max(CA[b], -(-nA // 128))
            CB[b] = max(CB[b], -(-nB // 128))
    C_A = [int(v) for v in CA]
    C_B = [int(v) for v in CB]
    # pad total chunks per block to even (pad chunk appended to B side)
    C_T = []
    for b in range(NB):
        t = C_A[b] + C_B[b]
        if t & 1:
            C_B[b] += 1
            t += 1
        C_T.append(t)
    OFF = np.concatenate([[0], np.cumsum(C_T)]).astype(np.int64)
    TOTC = int(OFF[-1])
    CBMAX = max(C_T)

    # fp8 wave layout of x (x/16), used only for host pre-gather of layer 0.
    xa = np.zeros((CORES * ASPL, D), F8)
    xb = np.zeros((CORES * BSPL, D), F8)
    for o in range(CORES):
        xs = (x[o * SHARD:(o + 1) * SHARD] / FSCALE).astype(F8)  # [1250, D]
        xa[o * ASPL:o * ASPL + ASPL] = xs[:ASPL]
        xb[o * BSPL:o * BSPL + SHARD - ASPL] = xs[ASPL:]

    Wd = np.ascontiguousarray(Ws.reshape(2 * N_LAYERS, D, D).astype(BF16))
    # b0 biases, feature-major column layout: bt[p, l*4+mc] = b0[l][mc*128+p]
    bT = np.ascontiguousarray(
        bs[:, 0].reshape(N_LAYERS, 4, 128).transpose(2, 0, 1).reshape(128, 4 * N_LAYERS))
    # b1 biases as K=1 matmul rows: [1, 3*512]
    b1r = np.ascontiguousarray(bs[:, 1].reshape(1, N_LAYERS * D).astype(BF16))
    ones1 = np.ones((1, 128), BF16)
    identb = np.eye(128, dtype=BF16)

    in_maps = []
    for c in range(CORES):
        Scnt = np.zeros((128, TOTC, 128), np.int16)
        idxd = np.zeros((128, TOTC * 8), np.int16)
        xgc = np.zeros((128, TOTC * D), F8)
        for b in range(NB):
            m = (owner == c) & (blk == b)
            eg = g[m]
            uniq, inv = np.unique(eg, return_inverse=True)
            ub = uniq % PADS >= ASPL
            nA = int((~ub).sum())
            nB = len(uniq) - nA
            cA = C_A[b]
            posmap = np.empty(len(uniq), np.int64)
            posmap[~ub] = np.arange(nA)
            posmap[ub] = cA * 128 + np.arange(nB)
            pos = posmap[inv]
            np.add.at(Scnt, (pos % 128, OFF[b] + pos // 128, j[m]), 1)
            # wave-buffer row ids for the unique rows
            u_rows = np.where(ub, (uniq // PADS) * BSPL + (uniq % PADS - ASPL),
                              (uniq // PADS) * ASPL + uniq % PADS).astype(np.int16)
            glist = np.zeros(C_T[b] * 128, np.int16)
            glist[:nA] = u_rows[~ub]
            glist[cA * 128:cA * 128 + nB] = u_rows[ub]
            w = glist.reshape(C_T[b] * 8, 16).T          # [16, C*8]
            idxd[:, OFF[b] * 8:(OFF[b] + C_T[b]) * 8] = np.tile(w, (8, 1))
            # layer-0 host pre-gather in exact tile layout
            gl = glist.astype(np.int64)
            rows = np.empty((C_T[b] * 128, D), F8)
            rows[:cA * 128] = xa[gl[:cA * 128]]
            rows[cA * 128:] = xb[gl[cA * 128:]]
            xgc[:, OFF[b] * D:(OFF[b] + C_T[b]) * D] = (
                rows.reshape(C_T[b], 128, D).transpose(1, 0, 2).reshape(128, C_T[b] * D))
        Sd = (Scnt.astype(np.float32) * FSCALE).astype(F8)
        # node-major bf16 x for this core, SBUF layout [128, NB*D]
        xn = np.zeros((NB, 128, D), np.float32)
        xn.reshape(-1, D)[:SHARD] = x[c * SHARD:(c + 1) * SHARD]
        hb = np.ascontiguousarray(
            xn.transpose(1, 0, 2).reshape(128, NB * D).astype(BF16))
        in_maps.append({
            "xgc": xgc,
            "hb16": hb,
            "Wd": Wd,
            "bT": bT,
            "b1r": b1r,
            "ones1": ones1,
            "identb": identb,
            "Sd": Sd,
            "idxd": idxd,
        })
    return in_maps, C_A, C_B, C_T, [int(v) for v in OFF], CBMAX


def build_program(C_A, C_B, C_T, OFF, CBMAX):
    import concourse.bacc as bacc
    import concourse.bass as bass
    import concourse.mybir as mybir
    import concourse.tile as tile

    dt = mybir.dt
    f32, bf16, f8, i16 = dt.float32, dt.bfloat16, dt.float8e4, dt.int16
    AF = mybir.ActivationFunctionType
    DR = mybir.MatmulPerfMode.DoubleRow
    TOTC = OFF[-1]

    nc = bacc.Bacc("TRN2", target_bir_lowering=False, debug=False,
                   enable_asserts=False, num_devices=CORES, num_swdge_queues=4)

    xgc = nc.dram_tensor("xgc", [128, TOTC * D], f8, kind="ExternalInput")
    hb16d = nc.dram_tensor("hb16", [128, NB * D], bf16, kind="ExternalInput")
    Wd = nc.dram_tensor("Wd", [2 * N_LAYERS, D, D], bf16, kind="ExternalInput")
    bTd = nc.dram_tensor("bT", [128, 4 * N_LAYERS], f32, kind="ExternalInput")
    b1rd = nc.dram_tensor("b1r", [1, N_LAYERS * D], bf16, kind="ExternalInput")
    ones1d = nc.dram_tensor("ones1", [1, 128], bf16, kind="ExternalInput")
    identbd = nc.dram_tensor("identb", [128, 128], bf16, kind="ExternalInput")
    Sdr = nc.dram_tensor("Sd", [128, TOTC, 128], f8, kind="ExternalInput")
    idxd = nc.dram_tensor("idxd", [128, TOTC * 8], i16, kind="ExternalInput")
    outd = nc.dram_tensor("out", [PADS, D], f32, kind="ExternalOutput")

    NCHUNK = [(0, 512), (512, 512), (1024, PADS - 1024)]

    with tile.TileContext(nc) as tc, ExitStack() as ctx:
        p_const = ctx.enter_context(tc.tile_pool(name="const", bufs=1))
        p_big = ctx.enter_context(tc.tile_pool(name="big", bufs=1))
        p_g = ctx.enter_context(tc.tile_pool(name="gth", bufs=10))
        p_z = ctx.enter_context(tc.tile_pool(name="z", bufs=2))
        p_hbf = ctx.enter_context(tc.tile_pool(name="hbf", bufs=2))
        p_ot = ctx.enter_context(tc.tile_pool(name="ot", bufs=2))
        p_aggps = ctx.enter_context(tc.tile_pool(name="aggps", bufs=2, space="PSUM"))
        p_tps = ctx.enter_context(tc.tile_pool(name="tps", bufs=2, space="PSUM"))
        p_mlpps = ctx.enter_context(tc.tile_pool(name="mlpps", bufs=2, space="PSUM"))
        p_dram = ctx.enter_context(tc.tile_pool(name="dram", bufs=1, space="DRAM"))

        # ---- constants (first-needed-first on each queue) ------------------
        identb = p_const.tile([128, 128], bf16)
        nc.sync.dma_start(identb[:], identbd.ap())

        # selector chunks on the scalar HWDGE queue, block order
        S = p_big.tile([128, TOTC, 128], f8)
        for b in range(2):
            nc.scalar.dma_start(S[:, OFF[b]:OFF[b + 1], :],
                                Sdr.ap()[:, OFF[b]:OFF[b + 1], :])

        hb16 = p_big.tile([128, NB, D], bf16)
        nc.sync.dma_start(hb16[:, :, :], hb16d.ap())

        bt = p_const.tile([128, 4 * N_LAYERS], f32)
        nc.scalar.dma_start(bt[:], bTd.ap())
        b1r = p_const.tile([1, N_LAYERS * D], bf16)
        nc.scalar.dma_start(b1r[:], b1rd.ap())
        ones1 = p_const.tile([1, 128], bf16)
        nc.scalar.dma_start(ones1[:], ones1d.ap())

        for b in range(2, NB):
            nc.scalar.dma_start(S[:, OFF[b]:OFF[b + 1], :],
                                Sdr.ap()[:, OFF[b]:OFF[b + 1], :])

        idxs = p_const.tile([128, TOTC * 8], i16)
        nc.scalar.dma_start(idxs[:], idxd.ap())

        Wts = {}
        for l in range(N_LAYERS):
            W0t = p_big.tile([128, 4, D], bf16)
            W1t = p_big.tile([128, 4, D], bf16)
            Wts[l] = (W0t, W1t)

        def emit_wload(l):
            W0t, W1t = Wts[l]
            for kc in range(4):
                nc.sync.dma_start(W0t[:, kc, :], Wd.ap()[2 * l, kc * 128:(kc + 1) * 128, :])
                nc.sync.dma_start(W1t[:, kc, :], Wd.ap()[2 * l + 1, kc * 128:(kc + 1) * 128, :])

        ZT = p_big.tile([128, 4, PADS], bf16)
        Y1T = p_big.tile([128, 4, PADS], bf16)

        # collective staging
        wa_in = p_dram.tile([128, D], bf16, name="wa_in")
        wa_out = p_dram.tile([128 * CORES, D], bf16, addr_space="Shared", name="wa_out")
        nc.sync.dma_start(wa_in[:, :], Wd.ap()[0, 0:128, 0:D])

        hsh_a = [p_dram.tile([ASPL, D], f8, name=f"hsa{l}") for l in range(2)]
        hsh_b = [p_dram.tile([BSPL, D], f8, name=f"hsb{l}") for l in range(2)]
        ag_a = [p_dram.tile([CORES * ASPL, D], f8, addr_space="Shared",
                            name=f"aga{l}") for l in range(2)]
        ag_b = [p_dram.tile([CORES * BSPL, D], f8, addr_space="Shared",
                            name=f"agb{l}") for l in range(2)]

        qctr = [0]
        gtiles = {}

        def emit_gather(l, b, half):
            # half 0: A rows [0, cA); half 1: B rows [cA, C_T)
            cA = C_A[b]
            cH = cA if half == 0 else C_T[b] - cA
            o = 0 if half == 0 else cA
            if half == 0:
                gt = p_g.tile([128, CBMAX, D], f8, tag="g", name="g")
                gtiles[(l, b)] = gt
            else:
                gt = gtiles[(l, b)]
            src = (ag_a[l - 1] if half == 0 else ag_b[l - 1])[:, :]
            qn = qctr[0] % 4
            qctr[0] += 1
            nc.gpsimd.dma_gather(
                out_ap=gt[:, o:o + cH, :],
                in_ap=src,
                idxs_ap=idxs[:, (OFF[b] + o) * 8:(OFF[b] + o + cH) * 8],
                num_idxs=cH * 128,
                num_idxs_reg=cH * 128,
                elem_size=D,
                single_packet=False,
                queue_num=qn,
            )

        def emit_load0(b):
            gt = p_g.tile([128, CBMAX, D], f8, tag="g", name="g")
            gtiles[(0, b)] = gt
            nc.sync.dma_start(gt[:, :C_T[b], :],
                              xgc.ap()[:, OFF[b] * D:(OFF[b] + C_T[b]) * D])

        def emit_agg(l, b):
            # selector matmuls -> PSUM; Z = psum + h (DVE); transpose Z -> ZT
            gt = gtiles.pop((l, b))
            NP = C_T[b] // 2
            ps = p_aggps.tile([128, D], f32, tag="agg", name="ps")
            for p in range(NP):
                nc.tensor.matmul(ps[:], lhsT=S[:, OFF[b] + 2 * p:OFF[b] + 2 * p + 2, :],
                                 rhs=gt[:, 2 * p:2 * p + 2, :],
                                 start=(p == 0), stop=(p == NP - 1),
                                 perf_mode=DR)
            zb = p_z.tile([128, D], bf16, tag="z", name="zb")
            nc.vector.tensor_add(zb[:], ps[:], hb16[:, b, :])
            for fc in range(4):
                pt = p_tps.tile([128, 128], bf16, tag="t", name="pt")
                nc.tensor.transpose(pt[:], zb[:, fc * 128:(fc + 1) * 128], identb[:])
                nc.vector.tensor_copy(ZT[:, fc, b * 128:(b + 1) * 128], pt[:])

        def emit_j0(l, c):
            nofs, nw = NCHUNK[c]
            W0t = Wts[l][0]
            for mc in range(4):
                ps2 = p_mlpps.tile([128, D], f32, tag="mlp", name="ps2")
                for kc in range(4):
                    nc.tensor.matmul(
                        ps2[:, :nw],
                        lhsT=W0t[:, kc, mc * 128:(mc + 1) * 128],
                        rhs=ZT[:, kc, nofs:nofs + nw],
                        start=(kc == 0), stop=(kc == 3))
                col = l * 4 + mc
                nc.scalar.activation(Y1T[:, mc, nofs:nofs + nw],
                                     ps2[:, :nw], AF.Relu, bias=bt[:, col:col + 1])

        def emit_j1(l, b):
            W1t = Wts[l][1]
            ps2 = p_mlpps.tile([128, D], f32, tag="mlp", name="ps2")
            nc.tensor.matmul(ps2[:], lhsT=ones1[:1, :],
                             rhs=b1r[:1, l * D:(l + 1) * D], start=True, stop=False)
            for kc in range(4):
                nc.tensor.matmul(ps2[:],
                                 lhsT=Y1T[:, kc, b * 128:(b + 1) * 128],
                                 rhs=W1t[:, kc, :],
                                 start=False, stop=(kc == 3))
            if l < N_LAYERS - 1:
                hf = p_hbf.tile([128, D], f8, tag="hbf", name="hf")
                nc.scalar.activation(hf[:], ps2[:], AF.Relu, scale=1.0 / FSCALE)
                if b < 4:
                    nc.sync.dma_start(hsh_a[l][b * 128:(b + 1) * 128, :], hf[:])
                else:
                    nc.sync.dma_start(hsh_b[l][(b - 4) * 128:(b - 3) * 128, :], hf[:])
                nc.scalar.activation(hb16[:, b, :], ps2[:], AF.Relu)
            else:
                ot = p_ot.tile([128, D], f32, tag="ot", name="ot")
                nc.scalar.activation(ot[:], ps2[:], AF.Identity)
                nc.sync.dma_start(outd.ap()[b * 128:(b + 1) * 128, :], ot[:])

        def emit_cc(ins, outs):
            nc.gpsimd.collective_compute(
                "AllGather", mybir.AluOpType.bypass,
                replica_groups=[list(range(CORES))],
                ins=[ins.opt()], outs=[outs.opt()])

        # ---- layer 0 loads + warmup collective -----------------------------
        for b in range(4):
            emit_load0(b)
        emit_wload(0)
        for b in range(4, NB):
            emit_load0(b)
        emit_wload(1)
        emit_wload(2)
        emit_cc(wa_in, wa_out)

        # ---- layers --------------------------------------------------------
        for l in range(N_LAYERS):
            for b in range(4):
                emit_agg(l, b)
            emit_j0(l, 0)
            for b in range(4, 8):
                emit_agg(l, b)
            emit_j0(l, 1)
            for b in range(4):
                emit_j1(l, b)
            if l < N_LAYERS - 1:
                emit_cc(hsh_a[l], ag_a[l])
            emit_agg(l, 8)
            emit_agg(l, 9)
            emit_j0(l, 2)
            for b in range(4, NB):
                emit_j1(l, b)
            if l < N_LAYERS - 1:
                emit_cc(hsh_b[l], ag_b[l])
                # next layer's gathers, A-calls lead
                order = []
                na, nb_ = 0, 0
                for k in range(2 * NB):
                    if na < NB and (na < 3 or na - nb_ < 4 or nb_ >= NB):
                        order.append((na, 0)); na += 1
                    else:
                        order.append((nb_, 1)); nb_ += 1
                for b, half in order:
                    emit_gather(l + 1, b, half)

    nc.compile()
    return nc


def kernel(**inputs):
    global LAST_RESULTS
    from concourse import bass_utils

    in_maps, C_A, C_B, C_T, OFF, CBMAX = _prep_host(
        inputs["x"], inputs["edge_index"], inputs["Ws"], inputs["bs"])
    nc = build_program(C_A, C_B, C_T, OFF, CBMAX)
    res = bass_utils.run_bass_kernel_spmd(
        nc, in_maps, core_ids=list(range(CORES)),
        trace=bool(int(os.environ.get("GIN_TRACE", "0"))),
        tmpdir=os.environ.get("GIN_TMPDIR"),
    )
    LAST_RESULTS = res
    out = np.empty((N_NODES, D), np.float32)
    for c in range(CORES):
        out[c * SHARD:(c + 1) * SHARD] = res.results[c]["out"][:SHARD]
    return out
